# revision 18
# baseline (speedup 1.0000x reference)
"""Trainium2 Bass kernel for nn_CapLayer_90056874263182.

Math note: the reference initializes routing logits b0 = zeros, so the
softmax over the 10 output caps starts uniform; s, v and delta_b are then
identical across caps, so the logits stay equal across caps through every
routing iteration and the softmax stays uniform forever.  The routing loop
therefore collapses exactly to

    v[b, o, :] = squash((1/10) * sum_i pred[b, i, :])   for every o

and  sum_i pred[b,i,:] = sum_{c,p} x[b,c,p] * W[c//8,:,p%8] + 144*sum_s Wb[s,:]

Kernel per core (64 batches), fp8 x / fp16 W data path (measured rel err
~8.5e-3 against the 2e-2 budget):
  - host relayouts the core's x shard to xt[cp, m, b] fp8-e4m3 where cp is the
    channel-pair (128 partitions), m = cl*144 + p enumerates the 288
    (channel-half, spatial) columns, b the 64 batches.  fp8 quarters HBM
    traffic; per-partition chunks stay contiguous (>= 512B descriptors).
  - PE does the whole contraction: for each m, one accumulating matmul
    with stationary xt[:, m, :] ([128, 64]) and moving W-block
    wsb[:, (m%8)*16 : ...] ([128, 16]) into PSUM S [64, 16]; weight loads
    are cheap and each matmul streams only 16 moving rows.  The bias row
    enters via a K=1 ones-matmul that runs as soon as consts land.
  - squash on ACT/DVE with the /10 folded into the activation scales;
    the output leaves via a SWDGE scatter-add prepared during the stream
    and fired by a Pool trigger (no HWDGE/DGE latency in the tail).
    The 10 identical caps are replicated host-side.
"""

import numpy as np

BS = 512          # full batch
NC = 8            # cores
B = BS // NC      # batches per core
CH = 256          # channels
HW = 144          # h*w
I8 = 8            # in_dim (= p % 8 bucket)
D = 16            # out_dim
NO = 10           # num output caps
M = 2 * HW        # 288 (cl, p) columns per channel-pair

# DMA chunks in m-columns.  Front-loaded big chunks keep the descriptor
# stream saturated; the short tail chunks shrink the post-last-byte
# matmul burst.
SUBS_M = [64, 64, 64, 48, 24, 16, 8]
assert sum(SUBS_M) == M


def _build_nc():
    from contextlib import ExitStack

    import concourse.bass as bass
    import concourse.mybir as mybir
    import concourse.tile as tile
    from concourse import bacc

    f32 = mybir.dt.float32
    f16 = mybir.dt.float16
    f8 = mybir.dt.float8e4
    AF = mybir.ActivationFunctionType

    # Bacc (not plain Bass): its finalize() runs the sync legalization
    # (event semaphores / matmul-wait moves) that splits multi-wait
    # instructions the TRN2 ISA can't encode.
    #
    # Bass.__init__ unconditionally memsets four const tiles on Pool before
    # the start barrier, delaying kernel start by ~340ns.  Three of them
    # (f32-1.0, bf16-1.0, u8-127) have no readers in this module (walrus
    # confirms), so skip their memsets; const-f32-0.0 is kept.
    orig_memset = bass.BassGpSimd.memset

    def _memset_skip_unused_consts(self, ap, value):
        name = getattr(getattr(ap, "tensor", None), "name", "") or ""
        if name.startswith("const-"):
            if name != "const-float32-0.0":
                return None
            # keep the one used const, but emit it on DVE: Pool is the
            # start-barrier laggard and its serial memset delays kernel start
            return self.bass.vector.memset(ap, value)
        return orig_memset(self, ap, value)

    bass.BassGpSimd.memset = _memset_skip_unused_consts
    try:
        nc = bacc.Bacc()
    finally:
        bass.BassGpSimd.memset = orig_memset
    # xt[cp, 8 + m, b] fp8, m = cl*144 + p so m % 8 == p % 8; the first
    # 8 m-slots (512 bytes/partition) carry the fp16 consts bit-packed so
    # chunk 0's single DMA delivers weights + x together: [:, :128] of the
    # bitcast = weight matrix, [0, 128:144] = bias row, [0, 144:208] = ones
    x = nc.dram_tensor("x", [128, (M + I8) * B], f8, kind="ExternalInput")
    # one row per batch, padded to 64 f32 (256B row stride — the SWDGE
    # scatter path needs a 256B-multiple DRAM step); host reads [:, :16].
    # The 10 identical caps are replicated host-side during the unshard.
    v = nc.dram_tensor("v", [B, 64], f32, kind="ExternalOutput")

    with tile.TileContext(nc) as tc, ExitStack() as ctx:
        consts = ctx.enter_context(tc.tile_pool(name="consts", bufs=1))
        xpool = ctx.enter_context(tc.tile_pool(name="xin", bufs=1))
        small = ctx.enter_context(tc.tile_pool(name="small", bufs=1))
        psum = ctx.enter_context(tc.tile_pool(name="psum", bufs=1, space="PSUM"))

        xts = []
        off = 0
        for t, s in enumerate(SUBS_M):
            s_eff = s + I8 if t == 0 else s  # chunk 0 carries the consts
            xt = xpool.tile([128, s_eff * B], f8, tag=f"xt{t}", bufs=1)
            nc.sync.dma_start(
                xt[:, :], x[:, off * B : (off + s_eff) * B]
            )
            xts.append(xt)
            off += s_eff
        wpk = xts[0][:, : I8 * B].bitcast(f16)
        # scatter-add token indices (row b -> v row b), packed in consts
        idxs = wpk[0:16, 224 : 224 + 4].bitcast(mybir.dt.int16)

        wsb = wpk[:, : I8 * D]
        bres = wpk[0:1, I8 * D : I8 * D + D]
        # ones row rides in the consts DMA: memsets would run on Pool ahead
        # of the start barrier and delay the whole kernel.
        ones = wpk[0:1, I8 * D + D : I8 * D + D + B]
        # Early ACT Sqrt warm-up on a consts element: places the (dep-free)
        # ACT table load ahead of the data-blocked Square in the ACT FIFO so
        # its 1283ns runs under the DMA stream, not in the tail.
        scr2 = consts.tile([1, 1], f32)
        nc.scalar.activation(scr2[:, :], wpk[0:1, 0:1], AF.Sqrt)

        # S[b, d] = brow[d] + sum_m xt[cp, m, b] * wsb[cp, (m%8)*16 + d]
        # brow enters via a K=1 ones-matmul as soon as the consts land.
        ps = psum.tile([B, D], f32)
        nc.tensor.matmul(ps[:, :], ones[:, :], bres[:, :], start=True, stop=False)
        m = 0
        for t, s in enumerate(SUBS_M):
            xv = xts[t][:, :].rearrange("c (m b) -> c m b", b=B)
            k0 = I8 if t == 0 else 0  # skip the consts slots in chunk 0
            for k in range(k0, k0 + s):
                i = (m + k - k0) % I8
                nc.tensor.matmul(
                    ps[:, :],
                    xv[:, k, :],
                    wsb[:, i * D : (i + 1) * D],
                    start=False,
                    stop=(m + k - k0 == M - 1),
                )
            m += s

        # Output staging [128, 64]: rows 0:64 cols 0:16 get v; everything
        # else stays zero (the scatter adds zeros to the padding).  The
        # memset runs early, hidden under the DMA stream.
        vpad = small.tile([128, 64], f32)
        nc.vector.memset(vpad[:, :], 0.0)
        # SWDGE prepare/trigger output path: descriptors are generated
        # during the stream (prep defers its vpad read to the trigger), so
        # after the squash only a Pool trigger + the 64x256B transfer +
        # DMA-sem prop remain -- no HWDGE (625ns) / DGE-delay (650ns).
        dma_sem = nc.alloc_semaphore("swdge_out")
        nc.gpsimd.dma_scatter_add(
            v[:, :],
            vpad[:, :].rearrange("p (one e) -> p one e", one=1),
            idxs[:, :],
            B,
            B,
            64,
            prepare_only=True,
            sem=dma_sem,
        )

        # squash with m = S/10 folded into the scales:
        #   nsq = |m|^2 = 0.01 * sum_d S^2,  rt = 0.1*sqrt(nsq),
        #   v_row = S * rt / (1 + nsq)
        sq = small.tile([B, D], f32)
        nsq = small.tile([B, 1], f32)
        nc.scalar.activation(
            sq[:, :], ps[:, :], AF.Square, scale=0.1, accum_out=nsq[:, :]
        )
        rt = small.tile([B, 1], f32)
        nc.scalar.activation(rt[:, :], nsq[:, :], AF.Sqrt, scale=0.01)
        # den/rec on DVE overlap the ACT Sqrt
        den = small.tile([B, 1], f32)
        nc.vector.tensor_scalar_add(den[:, :], nsq[:, :], 1.0)
        rec = small.tile([B, 1], f32)
        nc.vector.reciprocal(rec[:, :], den[:, :])

        # v_row = (S * rt) * rec in one dual-scalar DVE op
        nc.vector.tensor_scalar(
            vpad[0:B, 0:D],
            ps[:, :],
            rt[:, :],
            rec[:, :],
            op0=mybir.AluOpType.mult,
            op1=mybir.AluOpType.mult,
        )
        nc.gpsimd.trigger_dma(count=None)

    nc.finalize()

    # The no_exec timeline sim never bumps Tile's SWDGE completion lane
    # (DMASW0_*): on real hardware the SDMA engines increment it when the
    # scatter's descriptors drain, and the exec-mode interpreter mirrors
    # that, but neither path exists in the pure timeline sim — so the final
    # drain (which waits DMASW0 >= 16) would deadlock it.  Our swdge_out
    # sem IS fired in both modes (it is baked into the scatter descriptor,
    # and the sim's trigger track fires it after the modeled transfer +
    # sem-prop), so point the drain's wait at swdge_out instead.
    fn = nc.m.functions[0]
    out_sem = None
    for blk in fn.blocks:
        for inst in blk.instructions:
            si = inst.sync_info
            if si is None:
                continue
            for u in si.on_update:
                if u.ant_name == "swdge_out":
                    out_sem = u
    assert out_sem is not None
    drain_cluster = []
    sw_wait = None
    for blk in fn.blocks:
        for inst in blk.instructions:
            si = inst.sync_info
            if si is None:
                continue
            for w in si.on_wait:
                if (w.ant_name or "").startswith("DMASW"):
                    w.id = out_sem.id
                    w.ant_name = out_sem.ant_name
                    sw_wait = w
            if (
                type(inst).__name__ == "InstEventSemaphore"
                and si.on_wait
                and not si.on_update
                and any((w.ant_name or "").startswith("DMAHW") for w in si.on_wait)
            ):
                drain_cluster.append(inst)
    # The end drain is split into 2-wait chunks executed FIFO on SP; the
    # swdge_out wait (the last sem to fire) sits mid-cluster, serializing
    # the remaining 50ns chunks behind it.  Swap it into the final chunk
    # so the early waits retire during the scatter's sem propagation.
    if sw_wait is not None and drain_cluster:
        last = drain_cluster[-1]
        lw = last.sync_info.on_wait[-1]
        if lw is not sw_wait:
            for attr in ("id", "ant_name", "wait_mode", "wait_value"):
                tmp = getattr(lw, attr)
                setattr(lw, attr, getattr(sw_wait, attr))
                setattr(sw_wait, attr, tmp)
    return nc


def _host_inputs(x, W, Wb):
    x = np.ascontiguousarray(np.asarray(x, dtype=np.float32)).reshape(BS, CH, HW)
    W = np.asarray(W, dtype=np.float32)
    Wb = np.asarray(Wb, dtype=np.float32)

    # xt[core][cp, m, b] = x[64*core + b, 2*cp + m//144, m % 144], fp8 e4m3
    # (measured end-to-end rel err ~8.5e-3 vs the 2e-2 gate; W stays fp16)
    import ml_dtypes

    x16 = x.astype(ml_dtypes.float8_e4m3fn).reshape(NC, B, 128, 2 * HW)
    xcore = x16.transpose(0, 2, 3, 1)  # [NC, 128, 288, 64]

    # wsb[p, i*16 + d] = W[p//4, d, i]  (channel-pair p covers channels
    # 2p, 2p+1, both in group p//4; their shared weight is applied per
    # m-column, so no pre-summing is needed)
    wrj = np.empty((I8, 128, D), dtype=np.float32)
    s_of_p = np.arange(128) // 4
    for i in range(I8):
        wrj[i] = W[s_of_p, :, i]
    wrm = wrj.transpose(1, 0, 2).reshape(128, I8 * D)

    # packed consts [128, 256] fp16: cols :128 weights; row 0: cols 128:144
    # bias row, cols 144:208 ones.  brow[d] = 144 * sum_s Wb[s, d]  (the /10
    # happens in the ACT scale).  Bit-packed into the first 8*B fp8 slots of
    # each core's x tensor so one DMA carries consts + first x chunk.
    wr = np.zeros((128, 256), dtype=np.float32)
    wr[:, : I8 * D] = wrm
    wr[0, I8 * D : I8 * D + D] = HW * Wb.sum(axis=0)
    wr[0, I8 * D + D : I8 * D + D + B] = 1.0
    wr16 = wr.astype(np.float16)
    # scatter-add indices: idx i lives at [i % 16, i // 16], int16
    wr16[0:16, 224:228].view(np.int16)[:] = (
        np.arange(B, dtype=np.int16).reshape(4, 16).T
    )
    wr8 = wr16.view(np.uint8).view(ml_dtypes.float8_e4m3fn)
    xts = np.concatenate(
        [
            np.broadcast_to(wr8.reshape(1, 128, I8, B), (NC, 128, I8, B)),
            xcore,
        ],
        axis=2,
    )  # [NC, 128, 296, 64]
    return np.ascontiguousarray(xts), wr


def _run(x, W, Wb, trace=False):
    from concourse.bass_utils import run_bass_kernel_spmd

    xts, wr = _host_inputs(x, W, Wb)
    nc = _build_nc()
    in_maps = [
        {"x": np.ascontiguousarray(xts[k]), "wr": wr} for k in range(NC)
    ]
    res = run_bass_kernel_spmd(nc, in_maps, list(range(NC)), trace=trace)
    rows = np.concatenate(
        [res.results[k]["v"][:, :D] for k in range(NC)], axis=0
    )
    # unshard: replicate the (identical) caps into the full [BS, NO, D] shape
    out = np.ascontiguousarray(
        np.broadcast_to(rows.reshape(BS, 1, D), (BS, NO, D)), dtype=np.float32
    )
    return out, res


def _numpy_fallback(x, W, Wb, b0):
    """Generic routing on the host — only used if b0 is ever nonzero
    (the spec fills b0 with zeros, which collapses the routing; see top)."""
    x = np.asarray(x, np.float32)
    W = np.asarray(W, np.float32)
    Wb = np.asarray(Wb, np.float32)
    b0 = np.asarray(b0, np.float32)
    u = x.reshape(BS, 32, HW, I8)
    pred = np.einsum("bsni,soi->bsno", u, W) + Wb[None, :, None, :]
    pred = pred.reshape(BS, 32 * HW, D)
    b = np.broadcast_to(b0, (BS,) + b0.shape).copy()
    v = None
    for _ in range(3):
        e = np.exp(b - b.max(axis=1, keepdims=True))
        c = e / e.sum(axis=1, keepdims=True)
        s = np.einsum("boi,bid->bod", c, pred)
        nrm = np.linalg.norm(s, axis=2)
        coeff = (nrm * nrm / (1.0 + nrm * nrm)) / nrm
        v = s * coeff[:, :, None]
        b = b + np.einsum("bid,bod->boi", pred, v)
    return v.astype(np.float32)


def kernel(x, W, Wb, b0=None, **_ignored):
    if b0 is not None and np.any(np.asarray(b0)):
        return _numpy_fallback(x, W, Wb, b0)
    try:
        out, _ = _run(x, W, Wb, trace=False)
    except Exception:
        # one retry: the axon-tunneled device occasionally reports a
        # transient NRT_EXEC_UNIT_UNRECOVERABLE on first touch
        out, _ = _run(x, W, Wb, trace=False)
    return out


def kernel_traced(x, W, Wb, b0=None):
    """Like kernel() but also returns the BassKernelResults (exec_time_ns)."""
    return _run(x, W, Wb, trace=True)


# revision 19
# speedup vs baseline: 1.0088x; 1.0088x over previous
"""Trainium2 Bass kernel for nn_CapLayer_90056874263182.

Math note: the reference initializes routing logits b0 = zeros, so the
softmax over the 10 output caps starts uniform; s, v and delta_b are then
identical across caps, so the logits stay equal across caps through every
routing iteration and the softmax stays uniform forever.  The routing loop
therefore collapses exactly to

    v[b, o, :] = squash((1/10) * sum_i pred[b, i, :])   for every o

and  sum_i pred[b,i,:] = sum_{c,p} x[b,c,p] * W[c//8,:,p%8] + 144*sum_s Wb[s,:]

Kernel per core (64 batches), fp8 x / fp16 W data path (measured rel err
~8.5e-3 against the 2e-2 budget):
  - host relayouts the core's x shard to xt[cp, m, b] fp8-e4m3 where cp is the
    channel-pair (128 partitions), m = cl*144 + p enumerates the 288
    (channel-half, spatial) columns, b the 64 batches.  fp8 quarters HBM
    traffic; per-partition chunks stay contiguous (>= 512B descriptors).
  - PE does the whole contraction: for each m, one accumulating matmul
    with stationary xt[:, m, :] ([128, 64]) and moving W-block
    wsb[:, (m%8)*16 : ...] ([128, 16]) into PSUM S [64, 16]; weight loads
    are cheap and each matmul streams only 16 moving rows.  The bias row
    enters via a K=1 ones-matmul that runs as soon as consts land.
  - squash on ACT/DVE with the /10 folded into the activation scales;
    the output leaves via a SWDGE scatter-add prepared during the stream
    and fired by a Pool trigger (no HWDGE/DGE latency in the tail).
    The 10 identical caps are replicated host-side.
"""

import numpy as np

BS = 512          # full batch
NC = 8            # cores
B = BS // NC      # batches per core
CH = 256          # channels
HW = 144          # h*w
I8 = 8            # in_dim (= p % 8 bucket)
D = 16            # out_dim
NO = 10           # num output caps
M = 2 * HW        # 288 (cl, p) columns per channel-pair

# DMA chunks in m-columns.  Front-loaded big chunks keep the descriptor
# stream saturated; the short tail chunks shrink the post-last-byte
# matmul burst.
SUBS_M = [64, 64, 64, 48, 24, 16, 8]
assert sum(SUBS_M) == M


def _build_nc():
    from contextlib import ExitStack

    import concourse.bass as bass
    import concourse.mybir as mybir
    import concourse.tile as tile
    from concourse import bacc

    f32 = mybir.dt.float32
    f16 = mybir.dt.float16
    f8 = mybir.dt.float8e4
    AF = mybir.ActivationFunctionType

    # Bacc (not plain Bass): its finalize() runs the sync legalization
    # (event semaphores / matmul-wait moves) that splits multi-wait
    # instructions the TRN2 ISA can't encode.
    #
    # Bass.__init__ unconditionally memsets four const tiles on Pool before
    # the start barrier, delaying kernel start by ~340ns.  Three of them
    # (f32-1.0, bf16-1.0, u8-127) have no readers in this module (walrus
    # confirms), so skip their memsets; const-f32-0.0 is kept.
    orig_memset = bass.BassGpSimd.memset

    def _memset_skip_unused_consts(self, ap, value):
        name = getattr(getattr(ap, "tensor", None), "name", "") or ""
        if name.startswith("const-"):
            if name != "const-float32-0.0":
                return None
            # keep the one used const, but emit it on DVE: Pool is the
            # start-barrier laggard and its serial memset delays kernel start
            return self.bass.vector.memset(ap, value)
        return orig_memset(self, ap, value)

    bass.BassGpSimd.memset = _memset_skip_unused_consts
    try:
        nc = bacc.Bacc()
    finally:
        bass.BassGpSimd.memset = orig_memset
    # xt[cp, 8 + m, b] fp8, m = cl*144 + p so m % 8 == p % 8; the first
    # 8 m-slots (512 bytes/partition) carry the fp16 consts bit-packed so
    # chunk 0's single DMA delivers weights + x together: [:, :128] of the
    # bitcast = weight matrix, [0, 128:144] = bias row, [0, 144:208] = ones
    x = nc.dram_tensor("x", [128, (M + I8) * B], f8, kind="ExternalInput")
    # one row per batch, padded to 64 f32 (256B row stride — the SWDGE
    # scatter path needs a 256B-multiple DRAM step); host reads [:, :16].
    # The 10 identical caps are replicated host-side during the unshard.
    v = nc.dram_tensor("v", [B, 64], f32, kind="ExternalOutput")

    with tile.TileContext(nc) as tc, ExitStack() as ctx:
        consts = ctx.enter_context(tc.tile_pool(name="consts", bufs=1))
        xpool = ctx.enter_context(tc.tile_pool(name="xin", bufs=1))
        small = ctx.enter_context(tc.tile_pool(name="small", bufs=1))
        psum = ctx.enter_context(tc.tile_pool(name="psum", bufs=1, space="PSUM"))

        xts = []
        off = 0
        for t, s in enumerate(SUBS_M):
            s_eff = s + I8 if t == 0 else s  # chunk 0 carries the consts
            xt = xpool.tile([128, s_eff * B], f8, tag=f"xt{t}", bufs=1)
            nc.sync.dma_start(
                xt[:, :], x[:, off * B : (off + s_eff) * B]
            )
            xts.append(xt)
            off += s_eff
        wpk = xts[0][:, : I8 * B].bitcast(f16)
        # scatter-add token indices (row b -> v row b), packed in consts
        idxs = wpk[0:16, 224 : 224 + 4].bitcast(mybir.dt.int16)

        wsb = wpk[:, : I8 * D]
        bres = wpk[0:1, I8 * D : I8 * D + D]
        # ones row rides in the consts DMA: memsets would run on Pool ahead
        # of the start barrier and delay the whole kernel.
        ones = wpk[0:1, I8 * D + D : I8 * D + D + B]
        # Early ACT Sqrt warm-up on a consts element: places the (dep-free)
        # ACT table load ahead of the data-blocked Square in the ACT FIFO so
        # its 1283ns runs under the DMA stream, not in the tail.
        scr2 = consts.tile([1, 1], f32)
        nc.scalar.activation(scr2[:, :], wpk[0:1, 0:1], AF.Sqrt)

        # S[b, d] = brow[d] + sum_m xt[cp, m, b] * wsb[cp, (m%8)*16 + d]
        # brow enters via a K=1 ones-matmul as soon as the consts land.
        ps = psum.tile([B, D], f32)
        nc.tensor.matmul(ps[:, :], ones[:, :], bres[:, :], start=True, stop=False)
        m = 0
        for t, s in enumerate(SUBS_M):
            xv = xts[t][:, :].rearrange("c (m b) -> c m b", b=B)
            k0 = I8 if t == 0 else 0  # skip the consts slots in chunk 0
            for k in range(k0, k0 + s):
                i = (m + k - k0) % I8
                nc.tensor.matmul(
                    ps[:, :],
                    xv[:, k, :],
                    wsb[:, i * D : (i + 1) * D],
                    start=False,
                    stop=(m + k - k0 == M - 1),
                )
            m += s

        # Output staging [128, 64]: rows 0:64 cols 0:16 get v; everything
        # else stays zero (the scatter adds zeros to the padding).  The
        # memset runs early, hidden under the DMA stream.
        vpad = small.tile([128, 64], f32)
        nc.vector.memset(vpad[:, :], 0.0)
        # SWDGE prepare/trigger output path: descriptors are generated
        # during the stream (prep defers its vpad read to the trigger), so
        # after the squash only a Pool trigger + the 64x256B transfer +
        # DMA-sem prop remain -- no HWDGE (625ns) / DGE-delay (650ns).
        dma_sem = nc.alloc_semaphore("swdge_out")
        nc.gpsimd.dma_scatter_add(
            v[:, :],
            vpad[:, :].rearrange("p (one e) -> p one e", one=1),
            idxs[:, :],
            B,
            B,
            64,
            prepare_only=True,
            sem=dma_sem,
        )

        # squash with m = S/10 folded into the scales:
        #   nsq = |m|^2 = 0.01 * sum_d S^2,  rt = 0.1*sqrt(nsq),
        #   v_row = S * rt / (1 + nsq)
        # squash intermediates live in PSUM: ACT's modeled PSUM access
        # latency (172 cyc) beats SBUF (222 cyc) on the critical chain
        sq = psum.tile([B, D], f32, tag="sq")
        nsq = psum.tile([B, 1], f32, tag="nsq")
        nc.scalar.activation(
            sq[:, :], ps[:, :], AF.Square, scale=0.1, accum_out=nsq[:, :]
        )
        rt = psum.tile([B, 1], f32, tag="rt")
        nc.scalar.activation(rt[:, :], nsq[:, :], AF.Sqrt, scale=0.01)
        # den/rec on DVE overlap the ACT Sqrt
        den = small.tile([B, 1], f32)
        nc.vector.tensor_scalar_add(den[:, :], nsq[:, :], 1.0)
        rec = small.tile([B, 1], f32)
        nc.vector.reciprocal(rec[:, :], den[:, :])

        # v_row = (S * rt) * rec in one dual-scalar DVE op
        nc.vector.tensor_scalar(
            vpad[0:B, 0:D],
            ps[:, :],
            rt[:, :],
            rec[:, :],
            op0=mybir.AluOpType.mult,
            op1=mybir.AluOpType.mult,
        )
        nc.gpsimd.trigger_dma(count=None)

    nc.finalize()

    # The no_exec timeline sim never bumps Tile's SWDGE completion lane
    # (DMASW0_*): on real hardware the SDMA engines increment it when the
    # scatter's descriptors drain, and the exec-mode interpreter mirrors
    # that, but neither path exists in the pure timeline sim — so the final
    # drain (which waits DMASW0 >= 16) would deadlock it.  Our swdge_out
    # sem IS fired in both modes (it is baked into the scatter descriptor,
    # and the sim's trigger track fires it after the modeled transfer +
    # sem-prop), so point the drain's wait at swdge_out instead.
    fn = nc.m.functions[0]
    out_sem = None
    for blk in fn.blocks:
        for inst in blk.instructions:
            si = inst.sync_info
            if si is None:
                continue
            for u in si.on_update:
                if u.ant_name == "swdge_out":
                    out_sem = u
    assert out_sem is not None
    drain_cluster = []
    sw_wait = None
    for blk in fn.blocks:
        for inst in blk.instructions:
            si = inst.sync_info
            if si is None:
                continue
            for w in si.on_wait:
                if (w.ant_name or "").startswith("DMASW"):
                    w.id = out_sem.id
                    w.ant_name = out_sem.ant_name
                    sw_wait = w
            if (
                type(inst).__name__ == "InstEventSemaphore"
                and si.on_wait
                and not si.on_update
                and any((w.ant_name or "").startswith("DMAHW") for w in si.on_wait)
            ):
                drain_cluster.append(inst)
    # The end drain is split into 2-wait chunks executed FIFO on SP; the
    # swdge_out wait (the last sem to fire) sits mid-cluster, serializing
    # the remaining 50ns chunks behind it.  Swap it into the final chunk
    # so the early waits retire during the scatter's sem propagation.
    if sw_wait is not None and drain_cluster:
        last = drain_cluster[-1]
        lw = last.sync_info.on_wait[-1]
        if lw is not sw_wait:
            for attr in ("id", "ant_name", "wait_mode", "wait_value"):
                tmp = getattr(lw, attr)
                setattr(lw, attr, getattr(sw_wait, attr))
                setattr(sw_wait, attr, tmp)
    return nc


def _host_inputs(x, W, Wb):
    x = np.ascontiguousarray(np.asarray(x, dtype=np.float32)).reshape(BS, CH, HW)
    W = np.asarray(W, dtype=np.float32)
    Wb = np.asarray(Wb, dtype=np.float32)

    # xt[core][cp, m, b] = x[64*core + b, 2*cp + m//144, m % 144], fp8 e4m3
    # (measured end-to-end rel err ~8.5e-3 vs the 2e-2 gate; W stays fp16)
    import ml_dtypes

    x16 = x.astype(ml_dtypes.float8_e4m3fn).reshape(NC, B, 128, 2 * HW)
    xcore = x16.transpose(0, 2, 3, 1)  # [NC, 128, 288, 64]

    # wsb[p, i*16 + d] = W[p//4, d, i]  (channel-pair p covers channels
    # 2p, 2p+1, both in group p//4; their shared weight is applied per
    # m-column, so no pre-summing is needed)
    wrj = np.empty((I8, 128, D), dtype=np.float32)
    s_of_p = np.arange(128) // 4
    for i in range(I8):
        wrj[i] = W[s_of_p, :, i]
    wrm = wrj.transpose(1, 0, 2).reshape(128, I8 * D)

    # packed consts [128, 256] fp16: cols :128 weights; row 0: cols 128:144
    # bias row, cols 144:208 ones.  brow[d] = 144 * sum_s Wb[s, d]  (the /10
    # happens in the ACT scale).  Bit-packed into the first 8*B fp8 slots of
    # each core's x tensor so one DMA carries consts + first x chunk.
    wr = np.zeros((128, 256), dtype=np.float32)
    wr[:, : I8 * D] = wrm
    wr[0, I8 * D : I8 * D + D] = HW * Wb.sum(axis=0)
    wr[0, I8 * D + D : I8 * D + D + B] = 1.0
    wr16 = wr.astype(np.float16)
    # scatter-add indices: idx i lives at [i % 16, i // 16], int16
    wr16[0:16, 224:228].view(np.int16)[:] = (
        np.arange(B, dtype=np.int16).reshape(4, 16).T
    )
    wr8 = wr16.view(np.uint8).view(ml_dtypes.float8_e4m3fn)
    xts = np.concatenate(
        [
            np.broadcast_to(wr8.reshape(1, 128, I8, B), (NC, 128, I8, B)),
            xcore,
        ],
        axis=2,
    )  # [NC, 128, 296, 64]
    return np.ascontiguousarray(xts), wr


def _run(x, W, Wb, trace=False):
    from concourse.bass_utils import run_bass_kernel_spmd

    xts, wr = _host_inputs(x, W, Wb)
    nc = _build_nc()
    in_maps = [
        {"x": np.ascontiguousarray(xts[k]), "wr": wr} for k in range(NC)
    ]
    res = run_bass_kernel_spmd(nc, in_maps, list(range(NC)), trace=trace)
    rows = np.concatenate(
        [res.results[k]["v"][:, :D] for k in range(NC)], axis=0
    )
    # unshard: replicate the (identical) caps into the full [BS, NO, D] shape
    out = np.ascontiguousarray(
        np.broadcast_to(rows.reshape(BS, 1, D), (BS, NO, D)), dtype=np.float32
    )
    return out, res


def _numpy_fallback(x, W, Wb, b0):
    """Generic routing on the host — only used if b0 is ever nonzero
    (the spec fills b0 with zeros, which collapses the routing; see top)."""
    x = np.asarray(x, np.float32)
    W = np.asarray(W, np.float32)
    Wb = np.asarray(Wb, np.float32)
    b0 = np.asarray(b0, np.float32)
    u = x.reshape(BS, 32, HW, I8)
    pred = np.einsum("bsni,soi->bsno", u, W) + Wb[None, :, None, :]
    pred = pred.reshape(BS, 32 * HW, D)
    b = np.broadcast_to(b0, (BS,) + b0.shape).copy()
    v = None
    for _ in range(3):
        e = np.exp(b - b.max(axis=1, keepdims=True))
        c = e / e.sum(axis=1, keepdims=True)
        s = np.einsum("boi,bid->bod", c, pred)
        nrm = np.linalg.norm(s, axis=2)
        coeff = (nrm * nrm / (1.0 + nrm * nrm)) / nrm
        v = s * coeff[:, :, None]
        b = b + np.einsum("bid,bod->boi", pred, v)
    return v.astype(np.float32)


def kernel(x, W, Wb, b0=None, **_ignored):
    if b0 is not None and np.any(np.asarray(b0)):
        return _numpy_fallback(x, W, Wb, b0)
    try:
        out, _ = _run(x, W, Wb, trace=False)
    except Exception:
        # one retry: the axon-tunneled device occasionally reports a
        # transient NRT_EXEC_UNIT_UNRECOVERABLE on first touch
        out, _ = _run(x, W, Wb, trace=False)
    return out


def kernel_traced(x, W, Wb, b0=None):
    """Like kernel() but also returns the BassKernelResults (exec_time_ns)."""
    return _run(x, W, Wb, trace=True)
